# revision 14
# baseline (speedup 1.0000x reference)
"""Grouped per-channel Linear + ReLU on 8 TRN2 NeuronCores.

Problem: out[b,c,e] = relu(sum_s x[b,s,c] * W[c,s,e] + bias[c,e])
  x: (256, 2048, 32) f32, W: (32, 2048, 2048) f32, bias: (32, 2048) f32
  out: (256, 32, 2048) f32

Sharding: expert/channel parallel — core i computes channels [4i, 4i+4).
Each core runs 4 independent GEMMs of (256x2048)@(2048x2048) with the
contraction dim S on SBUF partitions. x is pre-transposed on the host to
(C, S, B) so both matmul operands stream from DRAM with contiguous rows.

Per channel: x slab (S,B) loaded once (1 MB DMA); W streamed in 2 MB
chunks (4 k-tiles x full E row) for DMA efficiency; all 8 PSUM banks hold
the (2 bt x 4 et) output block accumulating over 16 k-tiles. The bias is
folded in as an extra K=1 matmul (lhsT = ones(1,128), rhs = bias row).
ReLU happens during PSUM->SBUF eviction on the Scalar engine, and each
(bt, c) output row goes out as one 1 MB DMA.
"""

import os
import sys

for _p in ("/opt/trn_rl_repo", "/root/.axon_site/_ro/trn_rl_repo"):
    if os.path.isdir(_p) and _p not in sys.path:
        sys.path.insert(0, _p)

import numpy as np
import ml_dtypes

import concourse.bacc as bacc
import concourse.mybir as mybir
from concourse import tile
from concourse.bass_utils import run_bass_kernel_spmd

B, S, C, E = 256, 2048, 32, 2048
NCORES = 8
CPC = C // NCORES          # channels per core = 4
P = 128
KT = S // P                # 16 k-tiles
NBT = B // P               # 2 batch tiles
FREE = 512                 # matmul moving free dim (one PSUM bank of f32)
NET = E // FREE            # 4 e-tiles
KC = 8                     # k-tiles per W DMA chunk (4 MB chunks)

# matmul dtype: "bfloat16" (fast, ~2e-3 rel), "float32r" (~1.5e-4 rel,
# DMA-bound 2x slower), "float32" (exact, 4x slower compute-bound)
MM_DTYPE = os.environ.get("KERNEL_MM_DTYPE", "bfloat16")

_DTYPES = {
    "bfloat16": (mybir.dt.bfloat16, ml_dtypes.bfloat16),
    "float32r": (mybir.dt.float32r, np.float32),
    "float32": (mybir.dt.float32, np.float32),
}

_nc_cache = {}


def _build(mm_dtype: str):
    mm_dt, _ = _DTYPES[mm_dtype]
    nc = bacc.Bacc(None, target_bir_lowering=False)
    xt = nc.dram_tensor("xt", [CPC, S, B], mm_dt, kind="ExternalInput")
    w = nc.dram_tensor("w", [CPC, S, E], mm_dt, kind="ExternalInput")
    bias = nc.dram_tensor("bias", [CPC, E], mm_dt, kind="ExternalInput")
    out = nc.dram_tensor("out", [B, CPC, E], mybir.dt.float32, kind="ExternalOutput")

    with tile.TileContext(nc) as tc:
        XKC = 8        # k-tiles per x piece (2 pieces of 512 KB per channel)
        XP = KT // XKC
        with (
            tc.tile_pool(name="const", bufs=1) as const,
            tc.tile_pool(name="xpool", bufs=2 * XP) as xpool,
            tc.tile_pool(name="bpool", bufs=2) as bpool,
            tc.tile_pool(name="wpool", bufs=3) as wpool,
            tc.tile_pool(name="opool", bufs=3) as opool,
            tc.tile_pool(name="psum", bufs=NBT * NET, space="PSUM") as psum,
        ):
            ones = const.tile([1, P], mm_dt)
            nc.any.memset(ones[:], 1.0)
            zbias = const.tile([P, 1], mybir.dt.float32)
            nc.any.memset(zbias[:], 0.0)

            def x_piece_dma(eng, xp, c, i):
                eng.dma_start(
                    xp[:],
                    xt[c, i * XKC * P : (i + 1) * XKC * P, :].rearrange(
                        "(k p) b -> p k b", p=P
                    ),
                )

            # x pieces + bias per channel, created lazily so prefetches can be
            # emitted from inside the previous channel's compute loop.
            xtiles: dict[int, list] = {}
            btiles: dict[int, object] = {}
            self_toggle = [True]  # W-chunk HWDGE ring alternation state

            def prefetch_channel(c, eng):
                xtiles[c] = [
                    xpool.tile([P, XKC, B], mm_dt, name="xp") for _ in range(XP)
                ]
                for i in range(XP):
                    x_piece_dma(eng, xtiles[c][i], c, i)
                bsb = bpool.tile([1, E], mm_dt)
                eng.dma_start(bsb[:], bias[c : c + 1, :])
                btiles[c] = bsb

            for c in range(CPC):
                if c == 0:
                    # latency-critical first channel: x piece 0 rides the fast
                    # SP-HWDGE (sync) ring ahead of the W chunks it feeds
                    xtiles[0] = [
                        xpool.tile([P, XKC, B], mm_dt, name="xp")
                        for _ in range(XP)
                    ]
                    x_piece_dma(nc.sync, xtiles[0][0], 0, 0)
                    bsb = bpool.tile([1, E], mm_dt)
                    nc.gpsimd.dma_start(bsb[:], bias[0:1, :])
                    btiles[0] = bsb
                xps = xtiles[c]
                bsb = btiles[c]

                ps = [
                    [
                        psum.tile([P, FREE], mybir.dt.float32, name="ps")
                        for _ in range(NET)
                    ]
                    for _ in range(NBT)
                ]
                # W chunk schedule (k-tiles per DMA): ramp up at kernel start
                # so the first matmuls don't wait on a full 4 MB transfer.
                # Chunks alternate between the two HWDGE rings (SP "sync" and
                # ACT "scalar") so each transfer's fixed completion tail
                # overlaps the other ring's streaming.
                chunk_kts = [1, 1, 2, 4, 8] if c == 0 else [KC] * (KT // KC)
                k = 0
                prefetched = False
                for ci, ckt in enumerate(chunk_kts):
                    if c == 0 and k < XKC <= k + ckt:
                        # second x piece of channel 0, just ahead of its W
                        x_piece_dma(nc.sync, xtiles[0][1], 0, 1)
                    wsb = wpool.tile([P, KC, E], mm_dt, name="wsb")
                    weng = nc.sync if self_toggle[0] else nc.scalar
                    self_toggle[0] = not self_toggle[0]
                    weng.dma_start(
                        wsb[:, :ckt, :],
                        w[c, k * P : (k + ckt) * P, :].rearrange(
                            "(k p) e -> p k e", p=P
                        ),
                    )
                    for kk in range(ckt):
                        for bt in range(NBT):
                            xp = xps[k // XKC]
                            lhsT = xp[:, k % XKC, bt * P : (bt + 1) * P]
                            for et in range(NET):
                                nc.tensor.matmul(
                                    ps[bt][et][:],
                                    lhsT,
                                    wsb[:, kk, et * FREE : (et + 1) * FREE],
                                    start=(k == 0),
                                    stop=False,
                                )
                        k += 1
                    if not prefetched and k >= XKC and c + 1 < CPC:
                        # early prefetch of the next channel's x + bias on the
                        # SWDGE ring, spread out during this channel's compute
                        prefetch_channel(c + 1, nc.gpsimd)
                        prefetched = True
                # bias row: psum += ones(1,128).T @ bias(1,FREE)
                for bt in range(NBT):
                    for et in range(NET):
                        nc.tensor.matmul(
                            ps[bt][et][:],
                            ones[0:1, :],
                            bsb[0:1, et * FREE : (et + 1) * FREE],
                            start=False,
                            stop=True,
                        )
                # Evict with fused ReLU, mostly on VectorE (max with 0) with
                # ScalarE taking one subtile, so PSUM banks free up fast and
                # ACT stays available to trigger its HWDGE W chunks.
                last = c == CPC - 1
                for bt in range(NBT):
                    ot = opool.tile([P, E], mybir.dt.float32)
                    for et in range(NET):
                        dst = ot[:, et * FREE : (et + 1) * FREE]
                        if et == 0:
                            nc.scalar.activation(
                                dst,
                                ps[bt][et][:],
                                mybir.ActivationFunctionType.Relu,
                                bias=zbias[:],
                            )
                        else:
                            nc.vector.tensor_scalar_max(dst, ps[bt][et][:], 0.0)
                        if last:
                            # tail: small per-et DMAs so the final writes
                            # start as soon as each subtile is ready
                            nc.gpsimd.dma_start(
                                out[
                                    bt * P : (bt + 1) * P,
                                    c,
                                    et * FREE : (et + 1) * FREE,
                                ],
                                dst,
                            )
                    if not last:
                        # one 1 MB DMA per (bt, c) on the SWDGE ring, away
                        # from both W streams
                        nc.gpsimd.dma_start(out[bt * P : (bt + 1) * P, c, :], ot[:])
    nc.compile()
    return nc


def _get_nc(mm_dtype: str):
    if mm_dtype not in _nc_cache:
        _nc_cache[mm_dtype] = _build(mm_dtype)
    return _nc_cache[mm_dtype]


def _run(x, W, b, mm_dtype=None, **spmd_kwargs):
    mm_dtype = mm_dtype or MM_DTYPE
    _, np_dt = _DTYPES[mm_dtype]
    nc = _get_nc(mm_dtype)

    in_maps = []
    for i in range(NCORES):
        c0, c1 = i * CPC, (i + 1) * CPC
        xt_i = np.ascontiguousarray(
            x[:, :, c0:c1].transpose(2, 1, 0).astype(np_dt)
        )
        w_i = np.ascontiguousarray(W[c0:c1].astype(np_dt))
        b_i = np.ascontiguousarray(b[c0:c1].astype(np_dt))
        in_maps.append({"xt": xt_i, "w": w_i, "bias": b_i})

    res = run_bass_kernel_spmd(nc, in_maps, core_ids=list(range(NCORES)), **spmd_kwargs)
    out = np.concatenate([r["out"] for r in res.results], axis=1)
    return out, res


def kernel(x: np.ndarray, W: np.ndarray, b: np.ndarray) -> np.ndarray:
    out, _ = _run(x, W, b)
    return out


# revision 15
# speedup vs baseline: 1.0960x; 1.0960x over previous
"""Grouped per-channel Linear + ReLU on 8 TRN2 NeuronCores.

Problem: out[b,c,e] = relu(sum_s x[b,s,c] * W[c,s,e] + bias[c,e])
  x: (256, 2048, 32) f32, W: (32, 2048, 2048) f32, bias: (32, 2048) f32
  out: (256, 32, 2048) f32

Sharding: expert/channel parallel — core i computes channels [4i, 4i+4).
Each core runs 4 independent GEMMs of (256x2048)@(2048x2048) with the
contraction dim S on SBUF partitions. x is pre-transposed on the host to
(C, S, B) so both matmul operands stream from DRAM with contiguous rows.

Per channel: the x slab (S,B) is one 1 MB DMA (channel 0's rides the fast
SP-HWDGE ring ahead of the W chunks; later channels prefetch early on the
SWDGE ring). W streams on the SP-HWDGE ring in 2 MB chunks (4 k-tiles x
full E row). All 8 PSUM banks hold the (2 bt x 4 et) output block
accumulating over 16 k-tiles. The bias is folded in as an extra K=1
matmul (lhsT = ones(1,128), rhs = bias row). PSUM eviction applies ReLU
(ScalarE activation for one subtile, VectorE max-with-0 for the rest) and
outputs leave as 1 MB DMAs on the ACT-HWDGE ring (smaller eager DMAs for
the last channel to shorten the kernel tail).
"""

import os
import sys

for _p in ("/opt/trn_rl_repo", "/root/.axon_site/_ro/trn_rl_repo"):
    if os.path.isdir(_p) and _p not in sys.path:
        sys.path.insert(0, _p)

import numpy as np
import ml_dtypes

import concourse.bacc as bacc
import concourse.mybir as mybir
from concourse import tile
from concourse.bass_utils import run_bass_kernel_spmd

B, S, C, E = 256, 2048, 32, 2048
NCORES = 8
CPC = C // NCORES          # channels per core = 4
P = 128
KT = S // P                # 16 k-tiles
NBT = B // P               # 2 batch tiles
FREE = 512                 # matmul moving free dim (one PSUM bank of f32)
NET = E // FREE            # 4 e-tiles
KC = 4                     # k-tiles per W DMA chunk (2 MB chunks)

# matmul dtype: "bfloat16" (fast, ~2e-3 rel), "float32r" (~1.5e-4 rel,
# DMA-bound 2x slower), "float32" (exact, 4x slower compute-bound)
MM_DTYPE = os.environ.get("KERNEL_MM_DTYPE", "bfloat16")

_DTYPES = {
    "bfloat16": (mybir.dt.bfloat16, ml_dtypes.bfloat16),
    "float32r": (mybir.dt.float32r, np.float32),
    "float32": (mybir.dt.float32, np.float32),
}

_nc_cache = {}


def _build(mm_dtype: str):
    mm_dt, _ = _DTYPES[mm_dtype]
    nc = bacc.Bacc(None, target_bir_lowering=False)
    xt = nc.dram_tensor("xt", [CPC, S, B], mm_dt, kind="ExternalInput")
    w = nc.dram_tensor("w", [CPC, S, E], mm_dt, kind="ExternalInput")
    bias = nc.dram_tensor("bias", [CPC, E], mm_dt, kind="ExternalInput")
    out = nc.dram_tensor("out", [B, CPC, E], mybir.dt.float32, kind="ExternalOutput")

    with tile.TileContext(nc) as tc:
        with (
            tc.tile_pool(name="const", bufs=1) as const,
            tc.tile_pool(name="xpool", bufs=2) as xpool,
            tc.tile_pool(name="bpool", bufs=2) as bpool,
            tc.tile_pool(name="wpool", bufs=4) as wpool,
            tc.tile_pool(name="opool", bufs=3) as opool,
            tc.tile_pool(name="psum", bufs=NBT * NET, space="PSUM") as psum,
        ):
            ones = const.tile([1, P], mm_dt)
            nc.any.memset(ones[:], 1.0)
            zbias = const.tile([P, 1], mybir.dt.float32)
            nc.any.memset(zbias[:], 0.0)

            # x slab + bias per channel, created lazily so prefetches can be
            # emitted from inside the previous channel's compute loop.
            xtiles: dict[int, object] = {}
            btiles: dict[int, object] = {}

            def prefetch_channel(c, eng):
                xsb = xpool.tile([P, KT, B], mm_dt, name="xsb")
                eng.dma_start(
                    xsb[:], xt[c, :, :].rearrange("(k p) b -> p k b", p=P)
                )
                xtiles[c] = xsb
                bsb = bpool.tile([1, E], mm_dt, name="bsb")
                eng.dma_start(bsb[:], bias[c : c + 1, :])
                btiles[c] = bsb

            for c in range(CPC):
                if c == 0:
                    # latency-critical first channel: x slab rides the fast
                    # SP-HWDGE (sync) ring ahead of the W chunks it feeds
                    prefetch_channel(0, nc.sync)
                xsb = xtiles[c]
                bsb = btiles[c]

                ps = [
                    [
                        psum.tile([P, FREE], mybir.dt.float32, name="ps")
                        for _ in range(NET)
                    ]
                    for _ in range(NBT)
                ]
                # W chunk schedule (k-tiles per DMA): ramp up at kernel start
                # so the first matmuls don't wait on a full 2 MB transfer.
                chunk_kts = [1, 1, 2, 4, 4, 4] if c == 0 else [KC] * (KT // KC)
                k = 0
                prefetched = False
                for ckt in chunk_kts:
                    wsb = wpool.tile([P, KC, E], mm_dt, name="wsb")
                    nc.sync.dma_start(
                        wsb[:, :ckt, :],
                        w[c, k * P : (k + ckt) * P, :].rearrange(
                            "(k p) e -> p k e", p=P
                        ),
                    )
                    for kk in range(ckt):
                        for bt in range(NBT):
                            lhsT = xsb[:, k, bt * P : (bt + 1) * P]
                            for et in range(NET):
                                nc.tensor.matmul(
                                    ps[bt][et][:],
                                    lhsT,
                                    wsb[:, kk, et * FREE : (et + 1) * FREE],
                                    start=(k == 0),
                                    stop=False,
                                )
                        k += 1
                    if not prefetched and k >= 8 and c + 1 < CPC:
                        # prefetch next channel's x + bias on the SWDGE ring
                        # while this channel still has half its compute left
                        prefetch_channel(c + 1, nc.gpsimd)
                        prefetched = True
                # bias row: psum += ones(1,128).T @ bias(1,FREE)
                for bt in range(NBT):
                    for et in range(NET):
                        nc.tensor.matmul(
                            ps[bt][et][:],
                            ones[0:1, :],
                            bsb[0:1, et * FREE : (et + 1) * FREE],
                            start=False,
                            stop=True,
                        )
                # Evict with fused ReLU: ScalarE takes one subtile per bt,
                # VectorE (max with 0) the rest, so PSUM banks free up fast.
                last = c == CPC - 1
                for bt in range(NBT):
                    ot = opool.tile([P, E], mybir.dt.float32)
                    for et in range(NET):
                        dst = ot[:, et * FREE : (et + 1) * FREE]
                        if et == 0:
                            nc.scalar.activation(
                                dst,
                                ps[bt][et][:],
                                mybir.ActivationFunctionType.Relu,
                                bias=zbias[:],
                            )
                        else:
                            nc.vector.tensor_scalar_max(dst, ps[bt][et][:], 0.0)
                        if last:
                            # tail: small eager DMAs so the final writes start
                            # as soon as each subtile is ready
                            nc.scalar.dma_start(
                                out[
                                    bt * P : (bt + 1) * P,
                                    c,
                                    et * FREE : (et + 1) * FREE,
                                ],
                                dst,
                            )
                    if not last:
                        # one 1 MB DMA per (bt, c) on the ACT HWDGE ring,
                        # separate from the W stream
                        nc.scalar.dma_start(out[bt * P : (bt + 1) * P, c, :], ot[:])
    nc.compile()
    return nc


def _get_nc(mm_dtype: str):
    if mm_dtype not in _nc_cache:
        _nc_cache[mm_dtype] = _build(mm_dtype)
    return _nc_cache[mm_dtype]


def _run(x, W, b, mm_dtype=None, **spmd_kwargs):
    mm_dtype = mm_dtype or MM_DTYPE
    _, np_dt = _DTYPES[mm_dtype]
    nc = _get_nc(mm_dtype)

    in_maps = []
    for i in range(NCORES):
        c0, c1 = i * CPC, (i + 1) * CPC
        xt_i = np.ascontiguousarray(
            x[:, :, c0:c1].transpose(2, 1, 0).astype(np_dt)
        )
        w_i = np.ascontiguousarray(W[c0:c1].astype(np_dt))
        b_i = np.ascontiguousarray(b[c0:c1].astype(np_dt))
        in_maps.append({"xt": xt_i, "w": w_i, "bias": b_i})

    res = run_bass_kernel_spmd(nc, in_maps, core_ids=list(range(NCORES)), **spmd_kwargs)
    out = np.concatenate([r["out"] for r in res.results], axis=1)
    return out, res


def kernel(x: np.ndarray, W: np.ndarray, b: np.ndarray) -> np.ndarray:
    out, _ = _run(x, W, b)
    return out
